# revision 2
# baseline (speedup 1.0000x reference)
"""Trainium2 Bass kernel for nn_AverageAttention.

Computation (per batch element b, L=4096 tokens, D=1024):
    avg   = cumsum(x, axis=tokens) / (t+1)            # cumulative average
    h     = LayerNorm(avg) (gamma/beta folded into w1/b1 on host)
    inter = relu(h @ w1 + b1)
    avg_o = inter @ w2 + b2 + avg
    gates = concat(x, avg_o) @ wg + bg
    out   = sigmoid(gates[:D]) * x + sigmoid(gates[D:]) * avg_o

Sharding: data-parallel over batch B=8 -> one batch element per NeuronCore.

v2 design notes (vs the bf16 baseline):
 - m1/m2 (the two D x D matmuls) run in fp16: the PE upconverts fp16 to
   e10m11 exactly, so precision ~ the fp16 quantization of the operands.
 - m3 (the 2D x 2D gating matmul, 2/3 of PE flops) runs in fp8 e4m3 with
   perf_mode=DoubleRow: 2x contraction per pass (~1.8x faster). Activations
   are scaled x16 and weights x1024 into e4m3 range; the 2^-14 unscale rides
   the sigmoid's ACT scale. Sigmoid softens the fp8 gate noise; measured
   end-to-end rel err ~1.3e-2 (tolerance 2e-2).
 - cumsum is a single fp16 pass (tri-matmul + one-hot carry re-injection);
   fp16 upconversion is exact so only the initial fp16(x) rounding matters.
   This replaces the baseline's bf16 hi/lo double-pass.
 - Everything is feature-major: the host supplies x already transposed
   (fp16 for the combine, e4m3*16 for m3) and takes the output back
   transposed, so only h/avg need on-chip DMA transposes (the baseline
   transposed x in and the output back out).
 - rstd for LN is computed on the DVE (bit-trick seed + 2 Newton steps)
   instead of ACT Sqrt: Copy/Identity/Relu/Sigmoid all live in one ACT
   function-set but Sqrt does not, so using ACT Sqrt forced a ~1.3us
   function-table reload per switch (~42us/kernel in the baseline).
 - supertile NT=512 (matmul moving free dim) halves LDWEIGHTS count per
   token vs NT=256 and is the fp8-DoubleRow sweet spot.
"""

import numpy as np
import ml_dtypes

B, L, D = 8, 4096, 1024
P = 128
NT = 512            # tokens per supertile (matmul moving free dim)
SX = 16.0           # fp8 activation scale
SW = 1024.0         # fp8 weight scale

_CACHE = {}


def _build(L_=L, reps=1):
    from contextlib import ExitStack

    import concourse.mybir as mybir
    import concourse.tile as tile
    from concourse import bacc
    from concourse.bass import ds, ts

    f32 = mybir.dt.float32
    f16 = mybir.dt.float16
    fp8 = mybir.dt.float8e4
    u32 = mybir.dt.uint32
    FT = mybir.ActivationFunctionType
    OP = mybir.AluOpType
    DR = mybir.MatmulPerfMode.DoubleRow

    n_tiles = L_ // P
    n_st = L_ // NT
    SUB = NT // P       # 4 tiles per supertile
    KD = D // P         # 8 feature chunks for D
    KG = 2 * D // P     # 16 for the gating matmul
    H = D // 2          # 512: fp32 psum bank width
    USCALE = 1.0 / (SX * SW)

    nc = bacc.Bacc("TRN2", target_bir_lowering=False, debug=False, num_devices=8)

    xtok_d = nc.dram_tensor("xtok", [L_, D], f16, kind="ExternalInput").ap()
    xf16_d = nc.dram_tensor("xf16", [D, L_], f16, kind="ExternalInput").ap()
    xf8_d = nc.dram_tensor("xf8", [D, L_], fp8, kind="ExternalInput").ap()
    w1_d = nc.dram_tensor("w1f", [D, D], f16, kind="ExternalInput").ap()
    b1_d = nc.dram_tensor("b1f", [D], f32, kind="ExternalInput").ap()
    w2_d = nc.dram_tensor("w2f", [D, D], f16, kind="ExternalInput").ap()
    b2_d = nc.dram_tensor("b2f", [D], f32, kind="ExternalInput").ap()
    wg_d = nc.dram_tensor("wg8", [2 * D, 2 * D], fp8, kind="ExternalInput").ap()
    bg_d = nc.dram_tensor("bgf", [2 * D], f32, kind="ExternalInput").ap()
    tri_d = nc.dram_tensor("triu", [P, P], f16, kind="ExternalInput").ap()
    ones_d = nc.dram_tensor("onesr", [32, P], f16, kind="ExternalInput").ap()
    rec_d = nc.dram_tensor("recip", [P, n_tiles], f32, kind="ExternalInput").ap()
    out_d = nc.dram_tensor("out", [D, L_], f32, kind="ExternalOutput").ap()

    xf16_r = xf16_d.rearrange("(k p) l -> p k l", p=P)
    xf8_r = xf8_d.rearrange("(k p) l -> p k l", p=P)
    out_r = out_d.rearrange("(k p) l -> p k l", p=P)

    with tile.TileContext(nc) as tc, ExitStack() as ctx:
        wpool = ctx.enter_context(tc.tile_pool(name="weights", bufs=1))
        xpool = ctx.enter_context(tc.tile_pool(name="xin", bufs=3))
        mpool = ctx.enter_context(tc.tile_pool(name="mid", bufs=2))
        spool = ctx.enter_context(tc.tile_pool(name="stats", bufs=4))
        tpool = ctx.enter_context(tc.tile_pool(name="tacts", bufs=3))
        apool = ctx.enter_context(tc.tile_pool(name="acts", bufs=2))
        ipool = ctx.enter_context(tc.tile_pool(name="iacts", bufs=1))
        gpool = ctx.enter_context(tc.tile_pool(name="gates", bufs=4))
        cpool = ctx.enter_context(tc.tile_pool(name="comb", bufs=3))
        cumpool = ctx.enter_context(tc.tile_pool(name="cum", bufs=2, space="PSUM"))
        mmpool = ctx.enter_context(tc.tile_pool(name="mm", bufs=4, space="PSUM"))

        # ---- persistent weights / constants ----
        # Constants + small tensors + w1 first so phase A / m1 start
        # immediately; weights go on the scalar HWDGE queue so token loads
        # (sync queue) don't queue behind them.
        tri_sb = wpool.tile([P, P], f16)
        nc.scalar.dma_start(tri_sb[:], tri_d)
        ones_sb = wpool.tile([32, P], f16)
        nc.scalar.dma_start(ones_sb[:], ones_d)
        rec_sb = wpool.tile([P, n_tiles], f32)
        nc.scalar.dma_start(rec_sb[:], rec_d)
        b1_sb = wpool.tile([P, KD], f32)
        nc.scalar.dma_start(b1_sb[:], b1_d.rearrange("(f p) -> p f", p=P))
        b2_sb = wpool.tile([P, KD], f32)
        nc.scalar.dma_start(b2_sb[:], b2_d.rearrange("(f p) -> p f", p=P))
        bg_sb = wpool.tile([P, KG], f32)
        nc.scalar.dma_start(bg_sb[:], bg_d.rearrange("(f p) -> p f", p=P))
        w1_sb = wpool.tile([P, KD, D], f16)
        w1_r = w1_d.rearrange("(k p) m -> p k m", p=P)
        for k in range(0, KD, 4):
            nc.scalar.dma_start(w1_sb[:, k:k + 4, :], w1_r[:, k:k + 4, :])
        w2_sb = wpool.tile([P, KD, D], f16)
        w2_r = w2_d.rearrange("(k p) m -> p k m", p=P)
        for k in range(0, KD, 4):
            nc.scalar.dma_start(w2_sb[:, k:k + 4, :], w2_r[:, k:k + 4, :])
        wg_sb = wpool.tile([P, KG, 2 * D], fp8)
        wg_r = wg_d.rearrange("(k p) m -> p k m", p=P)
        for k in range(0, KG, 4):
            nc.scalar.dma_start(wg_sb[:, k:k + 4, :], wg_r[:, k:k + 4, :])
        carry = wpool.tile([32, D], f16)
        c15_sb = wpool.tile([P, 1], f32)
        nc.vector.memset(c15_sb[:], 1.5)
        magic_sb = wpool.tile([P, 1], u32)
        nc.vector.memset(
            magic_sb.bitcast(f32)[:],
            np.frombuffer(np.uint32(0x5F3759DF).tobytes(), dtype=np.float32)[0]
            .item())
        # preload the (single) ACT function set while the first input DMA is
        # in flight: Copy/Identity/Relu/Sigmoid all live in one set
        warm_sb = wpool.tile([P, 1], f32)
        for _ft in (FT.Copy, FT.Identity, FT.Relu, FT.Sigmoid):
            nc.scalar.activation(warm_sb[:], c15_sb[:], _ft,
                                 bias=c15_sb[:] if _ft != FT.Copy else 0.0)

        trir = tri_sb[:]
        onesr = ones_sb[:]

        def phase_a(acts, st, j, first):
            """Load tile, cumsum, LN; produce transposed fp16 activations.

            The serial carry chain stays off the DVE/ACT hot paths: cumsum
            matmuls (PE) -> carry copy (ACT, its only phase-A op) -> one-hot
            carry matmul (PE). Routing the carry through DVE adds ~10us/st
            of PE stall because the chain queues behind stats/combine ops in
            the DVE's strict FIFO.
            """
            hT, avT = acts
            gi = st * SUB + j
            x_t = xpool.tile([P, D], f16, tag="x", name="x_t")
            nc.sync.dma_start(x_t[:], xtok_d[ts(gi, P)])

            cps = cumpool.tile([P, D], f32, tag="cum", name="cps")
            for half in range(2):
                sl = ds(half * H, H)
                if not first:
                    nc.tensor.matmul(cps[:, sl], onesr, carry[:, sl],
                                     start=True, stop=False)
                nc.tensor.matmul(cps[:, sl], trir, x_t[:, sl],
                                 start=first, stop=True)
            # cumsum row 127 is the new running carry; PSUM reads must start
            # 32-aligned, so copy rows 96..127 and select row 31 in the
            # carry matmul via the one-hot-row stationary matrix.
            nc.scalar.copy(carry[:], cps[96:128, :])

            # avg (fp16) + row sums for LN stats. avg/h are produced on the
            # DVE (not ACT) so phase A injects no ACT work between m3's
            # psum-draining sigmoids; sq rides ACT Square (same function set
            # as Sigmoid) instead.
            ssum = spool.tile([P, 1], f32, tag="ssum", name="ssum")
            avg = mpool.tile([P, D], f16, tag="avg", name="avg")
            nc.vector.tensor_scalar(avg[:], cps[:], rec_sb[:, gi:gi + 1],
                                    0.0, OP.mult, OP.add,
                                    accum_out=ssum[:])
            sq = mpool.tile([P, D], f16, tag="sq", name="sq")
            ssq = spool.tile([P, 1], f32, tag="ssq", name="ssq")
            nc.vector.scalar_tensor_tensor(sq[:], avg[:], 1.0, avg[:],
                                           OP.mult, OP.mult,
                                           accum_out=ssq[:])
            mu = spool.tile([P, 1], f32, tag="mu", name="mu")
            nc.vector.tensor_scalar_mul(mu[:], ssum[:], 1.0 / D)
            mmu = spool.tile([P, 1], f32, tag="mmu", name="mmu")
            nc.vector.tensor_mul(mmu[:], mu[:], mu[:])
            mme = spool.tile([P, 1], f32, tag="mme", name="mme")
            nc.vector.tensor_scalar_add(mme[:], mmu[:], -1e-6)
            # vare = var + eps = ssq/D - (mu^2 - eps)
            vare = spool.tile([P, 1], f32, tag="vare", name="vare")
            nc.vector.scalar_tensor_tensor(vare[:], ssq[:], 1.0 / D, mme[:],
                                           OP.mult, OP.subtract)
            # rstd = 1/sqrt(vare): bit-trick seed + 2 Newton iterations (DVE
            # only -- ACT Sqrt lives in a different function set than Sigmoid
            # and would force a table reload every switch)
            shif = spool.tile([P, 1], u32, tag="shif", name="shif")
            nc.vector.tensor_scalar(shif[:], vare.bitcast(u32)[:], 1, None,
                                    OP.logical_shift_right)
            seed = spool.tile([P, 1], u32, tag="seed", name="seed")
            nc.vector.tensor_sub(seed[:], magic_sb[:], shif[:])
            sf = seed.bitcast(f32)
            r2 = spool.tile([P, 1], f32, tag="r2", name="r2")
            nc.vector.tensor_mul(r2[:], sf[:], sf[:])
            p1 = spool.tile([P, 1], f32, tag="p1", name="p1")
            nc.vector.tensor_mul(p1[:], r2[:], vare[:])
            q1 = spool.tile([P, 1], f32, tag="q1", name="q1")
            nc.vector.scalar_tensor_tensor(q1[:], p1[:], -0.5, c15_sb[:],
                                           OP.mult, OP.add)
            r1 = spool.tile([P, 1], f32, tag="r1", name="r1")
            nc.vector.tensor_mul(r1[:], sf[:], q1[:])
            r2b = spool.tile([P, 1], f32, tag="r2b", name="r2b")
            nc.vector.tensor_mul(r2b[:], r1[:], r1[:])
            p2 = spool.tile([P, 1], f32, tag="p2", name="p2")
            nc.vector.tensor_mul(p2[:], r2b[:], vare[:])
            q2 = spool.tile([P, 1], f32, tag="q2", name="q2")
            nc.vector.scalar_tensor_tensor(q2[:], p2[:], -0.5, c15_sb[:],
                                           OP.mult, OP.add)
            rstd = spool.tile([P, 1], f32, tag="rstd", name="rstd")
            nc.vector.tensor_mul(rstd[:], r1[:], q2[:])
            nmr = spool.tile([P, 1], f32, tag="nmr", name="nmr")
            nc.vector.scalar_tensor_tensor(nmr[:], mu[:], -1.0, rstd[:],
                                           OP.mult, OP.mult)
            h_tm = mpool.tile([P, D], f16, tag="h_tm", name="h_tm")
            nc.vector.tensor_scalar(h_tm[:], avg[:], rstd[:], nmr[:],
                                    OP.mult, OP.add)

            # batched xbar transposes: [128, 1024] -> [128, 8, 128]
            tsl = ds(j * P, P)
            nc.sync.dma_start_transpose(hT[:, :, tsl], h_tm[:])
            nc.sync.dma_start_transpose(avT[:, :, tsl], avg[:])

        def alloc_trans(st):
            hT = tpool.tile([P, KD, NT], f16, tag="hT", name="hT")
            avT = tpool.tile([P, KD, NT], f16, tag="avT", name="avT")
            return hT, avT

        def alloc_x(st):
            x8 = apool.tile([P, KD, NT], fp8, tag="x8", name="x8")
            nc.sync.dma_start(x8[:], xf8_r[:, :, ts(st, NT)])
            xf = apool.tile([P, KD, NT], f16, tag="xf", name="xf")
            nc.sync.dma_start(xf[:], xf16_r[:, :, ts(st, NT)])
            return x8, xf

        def phase_m1(acts):
            hT, _ = acts
            inT = ipool.tile([P, KD, NT], f16, tag="inT", name="inT")
            for f in range(KD):
                ps = mmpool.tile([P, NT], f32, tag="mm", name="ps")
                for k in range(KD):
                    nc.tensor.matmul(ps[:], w1_sb[:, k, ds(f * P, P)],
                                     hT[:, k, :],
                                     start=(k == 0), stop=(k == KD - 1))
                nc.scalar.activation(inT[:, f, :], ps[:], FT.Relu,
                                     bias=b1_sb[:, f:f + 1])
            return inT

        def phase_m2(acts, inT):
            _, avT = acts
            aoT = ipool.tile([P, KD, NT], f16, tag="aoT", name="aoT")
            ao8 = ipool.tile([P, KD, NT], fp8, tag="ao8", name="ao8")
            for f in range(KD):
                ps = mmpool.tile([P, NT], f32, tag="mm", name="ps")
                for k in range(KD):
                    nc.tensor.matmul(ps[:], w2_sb[:, k, ds(f * P, P)],
                                     inT[:, k, :],
                                     start=(k == 0), stop=(k == KD - 1))
                nc.vector.scalar_tensor_tensor(aoT[:, f, :], ps[:],
                                               b2_sb[:, f:f + 1], avT[:, f, :],
                                               OP.add, OP.add)
                nc.vector.tensor_scalar_mul(ao8[:, f, :], aoT[:, f, :], SX)
            return aoT, ao8

        def phase_m3(xacts, aoT, ao8, st, hooks=None):
            x8, xf = xacts
            # chunk order pairs (c, c+KD) so each combine chunk can start as
            # soon as its input/forget gate chunks are done
            for c in range(KD):
                if hooks and c in hooks:
                    hooks[c]()
                sgs = []
                for cc in (c, c + KD):
                    ps = mmpool.tile([P, NT], f32, tag="mm", name="ps")
                    for kp in range(KD // 2):
                        nc.tensor.matmul(ps[:],
                                         wg_sb[:, 2 * kp:2 * kp + 2,
                                               ds(cc * P, P)],
                                         x8[:, 2 * kp:2 * kp + 2, :],
                                         start=(kp == 0), stop=False,
                                         perf_mode=DR)
                    for kp in range(KD // 2):
                        nc.tensor.matmul(ps[:],
                                         wg_sb[:, KD + 2 * kp:KD + 2 * kp + 2,
                                               ds(cc * P, P)],
                                         ao8[:, 2 * kp:2 * kp + 2, :],
                                         start=False, stop=(kp == KD // 2 - 1),
                                         perf_mode=DR)
                    sg = gpool.tile([P, NT], f16, tag="sg", name="sg")
                    nc.scalar.activation(sg[:], ps[:], FT.Sigmoid,
                                         scale=USCALE,
                                         bias=bg_sb[:, cc:cc + 1])
                    sgs.append(sg)
                t1 = cpool.tile([P, NT], f16, tag="t1", name="t1")
                t2 = cpool.tile([P, NT], f16, tag="t2", name="t2")
                oc = cpool.tile([P, NT], f32, tag="oc", name="oc")
                nc.vector.tensor_mul(t1[:], sgs[0][:], xf[:, c, :])
                nc.vector.tensor_mul(t2[:], sgs[1][:], aoT[:, c, :])
                nc.vector.tensor_add(oc[:], t1[:], t2[:])
                # out stores go on the sync queue: on the scalar queue their
                # descriptor dispatch delays the relu/sigmoid issue and
                # stalls the PE at the m1->m2 boundary
                nc.sync.dma_start(out_r[:, c, ts(st, NT)], oc[:])

        # software pipeline, flattened across reps and two supertiles deep:
        # phase A of (global) supertile g+2 interleaves with the matmul
        # phases of supertile g, so the h/avg transposes of g+1 are done a
        # full supertile before m1(g+1) needs them -- the DVE-FIFO latency of
        # the stats chain then never reaches the PE.
        n_g = reps * n_st
        trans = {0: alloc_trans(0)}
        for j in range(SUB):
            phase_a(trans[0], 0, j, first=(j == 0))
        xs = {0: alloc_x(0)}
        if n_g > 1:
            trans[1] = alloc_trans(1 % n_st)
            for j in range(SUB):
                phase_a(trans[1], 1 % n_st, j, first=False)
        for g in range(n_g):
            st = g % n_st
            if g + 1 < n_g:
                xs[g + 1] = alloc_x((g + 1) % n_st)
            nt = None
            if g + 2 < n_g:
                st2 = (g + 2) % n_st
                nt = alloc_trans(st2)
                trans[g + 2] = nt
                phase_a(nt, st2, 0, first=(st2 == 0))
            inT = phase_m1(trans[g])
            if nt is not None:
                phase_a(nt, st2, 1, first=False)
            aoT, ao8 = phase_m2(trans[g], inT)
            if nt is not None:
                phase_a(nt, st2, 2, first=False)
            # phase A of tile 3 goes late in the m3 chunk loop: only c6/c7's
            # matmuls sit behind a potential carry-chain stall at the PE
            # FIFO head
            hooks = None
            if nt is not None:
                hooks = {6: lambda: phase_a(nt, st2, 3, first=False)}
            phase_m3(xs[g], aoT, ao8, st, hooks)
            trans.pop(g, None)
            xs.pop(g, None)

    nc.compile()
    return nc


def _make_runner(nc, n_cores=8):
    """Build a cached jitted shard_map executor for the compiled Bass module
    (mirrors concourse.bass2jax.run_bass_via_pjrt, but reusable)."""
    import jax
    import concourse.mybir as mybir
    from concourse import bass2jax
    from jax.experimental.shard_map import shard_map
    from jax.sharding import Mesh, PartitionSpec

    bass2jax.install_neuronx_cc_hook()

    partition_name = (nc.partition_id_tensor.name
                      if nc.partition_id_tensor else None)
    in_names, out_names, out_avals, zero_outs = [], [], [], []
    for alloc in nc.m.functions[0].allocations:
        if not isinstance(alloc, mybir.MemoryLocationSet):
            continue
        name = alloc.memorylocations[0].name
        if alloc.kind == "ExternalInput":
            if name != partition_name:
                in_names.append(name)
        elif alloc.kind == "ExternalOutput":
            out_names.append(name)
            shape = tuple(alloc.tensor_shape)
            dtype = mybir.dt.np(alloc.dtype)
            out_avals.append(jax.core.ShapedArray(shape, dtype))
            zero_outs.append(np.zeros(shape, dtype))
    n_params = len(in_names)
    n_outs = len(out_avals)
    all_names = in_names + out_names
    if partition_name is not None:
        all_names = all_names + [partition_name]

    def _body(*args):
        operands = list(args)
        if partition_name is not None:
            operands.append(bass2jax.partition_id_tensor())
        outs = bass2jax._bass_exec_p.bind(
            *operands,
            out_avals=tuple(out_avals),
            in_names=tuple(all_names),
            out_names=tuple(out_names),
            lowering_input_output_aliases=(),
            sim_require_finite=True,
            sim_require_nnan=True,
            nc=nc,
        )
        return tuple(outs)

    devices = jax.devices()[:n_cores]
    mesh = Mesh(np.asarray(devices), ("core",))
    in_specs = (PartitionSpec("core"),) * (n_params + n_outs)
    out_specs = (PartitionSpec("core"),) * n_outs
    donate = tuple(range(n_params, n_params + n_outs))
    sharded = jax.jit(
        shard_map(_body, mesh=mesh, in_specs=in_specs, out_specs=out_specs,
                  check_rep=False),
        donate_argnums=donate, keep_unused=True,
    )

    def _concat(in_maps):
        concat_in = [
            np.concatenate([np.asarray(m[name]) for m in in_maps], axis=0)
            for name in in_names
        ]
        concat_zeros = [
            np.zeros((n_cores * z.shape[0], *z.shape[1:]), z.dtype)
            for z in zero_outs
        ]
        return concat_in, concat_zeros

    def run(in_maps):
        concat_in, concat_zeros = _concat(in_maps)
        out_arrs = sharded(*concat_in, *concat_zeros)
        return [
            {name: np.asarray(out_arrs[i]).reshape(n_cores, *out_avals[i].shape)[c]
             for i, name in enumerate(out_names)}
            for c in range(n_cores)
        ]

    def make_timed(in_maps):
        """Non-donating variant with device-resident inputs, for timing."""
        from jax.sharding import NamedSharding
        sharded_nd = jax.jit(
            shard_map(_body, mesh=mesh, in_specs=in_specs,
                      out_specs=out_specs, check_rep=False),
            keep_unused=True,
        )
        concat_in, concat_zeros = _concat(in_maps)
        sh = NamedSharding(mesh, PartitionSpec("core"))
        dev_args = [jax.device_put(a, sh) for a in concat_in + concat_zeros]
        jax.block_until_ready(dev_args)

        def timed_once():
            outs = sharded_nd(*dev_args)
            jax.block_until_ready(outs)
            return outs

        return timed_once

    run.make_timed = make_timed
    return run


def _prep_shared(w1, b1, w2, b2, ln_g, ln_b, wg, bg, L_=L):
    f16 = np.float16
    e4 = ml_dtypes.float8_e4m3
    w1g = (np.asarray(w1, np.float32) * np.asarray(ln_g, np.float32)[:, None])
    b1f = (np.asarray(ln_b, np.float64) @ np.asarray(w1, np.float64)
           + np.asarray(b1, np.float64)).astype(np.float32)
    wg8 = np.clip(np.asarray(wg, np.float32) * SW, -240.0, 240.0)
    shared = {
        "w1f": np.ascontiguousarray(w1g.astype(f16)),
        "b1f": b1f,
        "w2f": np.ascontiguousarray(np.asarray(w2, np.float32).astype(f16)),
        "b2f": np.asarray(b2, np.float32),
        "wg8": np.ascontiguousarray(wg8.astype(e4)),
        "bgf": np.asarray(bg, np.float32),
        "triu": np.triu(np.ones((P, P), np.float32)).astype(f16),
        "onesr": ((np.arange(32) == 31).astype(np.float32)[:, None]
                  .repeat(P, 1)).astype(f16),
        "recip": np.ascontiguousarray(
            (1.0 / (1.0 + np.arange(L_, dtype=np.float64)))
            .astype(np.float32).reshape(L_ // P, P).T),
    }
    return shared


def _prep_x(xb):
    """Per-batch input layouts: token-major fp16 (cumsum), feature-major fp16
    (combine), feature-major e4m3*16 (gating matmul)."""
    f16 = np.float16
    e4 = ml_dtypes.float8_e4m3
    xb = np.asarray(xb, np.float32)
    xT = np.ascontiguousarray(xb.T)
    return {
        "xtok": xb.astype(f16),
        "xf16": xT.astype(f16),
        "xf8": np.clip(xT * SX, -240.0, 240.0).astype(e4),
    }


def _get_runner(L_=L):
    key = ("runner", L_)
    if key not in _CACHE:
        nc = _build(L_)
        _CACHE[key] = _make_runner(nc)
    return _CACHE[key]


def kernel(inputs, w1, b1, w2, b2, ln_g, ln_b, wg, bg):
    inputs = np.asarray(inputs, dtype=np.float32)
    Bi, Li, Di = inputs.shape
    assert (Bi, Li, Di) == (B, L, D), (Bi, Li, Di)
    run = _get_runner(L)
    shared = _prep_shared(w1, b1, w2, b2, ln_g, ln_b, wg, bg, L)
    in_maps = [dict(shared, **_prep_x(inputs[b])) for b in range(B)]
    results = run(in_maps)
    return np.stack([np.ascontiguousarray(results[b]["out"].T)
                     for b in range(B)], axis=0)
